# revision 5
# baseline (speedup 1.0000x reference)
"""Bass/Trainium2 kernel for 9x9 bilateral denoising (edge-preserving blend).

Reference computation (per pixel, 9x9 neighborhood, C=3):
    dist  = sum_c (p_c - x_c)^2
    w     = exp(-50 * dist) * gauss2d(sigma=3)
    out   = clip(0.8 * x + 0.2 * (sum w*p / sum w), 0, 1)

Sharding: pure data parallel. 8 cores = 2 images x 4 horizontal bands of 128
rows. Host pre-pads each image by 4 (reflect), builds planar fp16 [3,137,520]
band tensors (rows 128v..128v+135 of the padded image) so every one of the 81
taps is a (partition-aligned row-shift via 9 preloaded tiles) x (free-dim
column slice). A column-shifted copy keeps fp16 operands 4-byte aligned for
the DVE 2x mode on odd column offsets.

Per-core device program (per tap): DVE fp16 diff -> ACT Square (fp32) ->
GPSIMD dist adds -> ACT exp(-50 d + ln g) -> DVE w*p (broadcast) -> TensorE
identity-matmul accumulate into PSUM (num_R,num_G,num_B,den). Finish: DVE
reciprocal, muls, fused blend + clip, DMA out.
"""

import math
import os
import sys

for _p in ("/opt/trn_rl_repo",):
    if _p not in sys.path:
        sys.path.insert(0, _p)

import numpy as np

import concourse.mybir as mybir
from concourse import bacc, bass_utils
from concourse.masks import make_identity
from concourse.tile import TileContext

# ---------------------------------------------------------------- constants
N_CORES = 8
B, H, W, C = 2, 512, 512, 3
KER = 9
PAD = 4
BAND = 128  # rows per core
PW = W + 2 * PAD  # padded width 520
SIGMA_COLOR = 0.1
SIGMA_SPACE = 3.0
EDGE_PRESERVE = 0.8
EXP_SCALE = -0.5 / (SIGMA_COLOR**2)  # -50.0

F16 = mybir.dt.float16
F32 = mybir.dt.float32


def _gauss2d():
    ax = np.arange(KER, dtype=np.float64) - (KER - 1) / 2.0
    g = np.exp(-0.5 * (ax / SIGMA_SPACE) ** 2)
    g = g / g.sum()
    k2d = np.outer(g, g)
    return k2d.astype(np.float64)  # [9, 9]


GAUSS2D = _gauss2d()


# ---------------------------------------------------------------- program
def build_program(n_reps=1, cfg=None):
    """Build + compile the single-core SPMD program. Returns nc."""
    cfg = dict(cfg or {})
    bcast_mul = cfg.get("bcast_mul", True)
    adds_gps = cfg.get("adds_gps", (True, True))  # (add1, add2) on gpsimd?
    sq_engines = cfg.get("sq_engines", "act")  # 'act' or 'act2dve1'

    nc = bacc.Bacc(
        "TRN2", target_bir_lowering=False, debug=False, num_devices=N_CORES
    )

    pe_d = nc.dram_tensor("pe", [C, BAND + KER, PW], F16, kind="ExternalInput")
    po_d = nc.dram_tensor("po", [C, BAND + KER, PW], F16, kind="ExternalInput")
    x32_d = nc.dram_tensor("x32", [BAND, C * W], F32, kind="ExternalInput")
    out_d = nc.dram_tensor("out", [BAND, C * W], F32, kind="ExternalOutput")

    with TileContext(nc) as tc:
        with (
            tc.tile_pool(name="persist", bufs=1) as pp,
            tc.tile_pool(name="pipe", bufs=4) as pipe,
            tc.tile_pool(name="psum", bufs=1, space="PSUM") as psp,
        ):
            # ---- persistent loads
            pe_rows = pe_d.ap().rearrange("c r w -> r c w")  # [137, 3, 520]
            po_rows = po_d.ap().rearrange("c r w -> r c w")

            te = []
            to = []
            # center tile first so taps can start ASAP
            order = [4, 0, 1, 2, 3, 5, 6, 7, 8]
            te_map = {}
            to_map = {}
            for i in order:
                t = pp.tile([BAND, C * PW], F16, tag=f"te{i}")
                nc.sync.dma_start(
                    out=t[:].rearrange("p (c w) -> p c w", c=C),
                    in_=pe_rows[i : i + BAND],
                )
                te_map[i] = t
                t = pp.tile([BAND, C * PW], F16, tag=f"to{i}")
                nc.sync.dma_start(
                    out=t[:].rearrange("p (c w) -> p c w", c=C),
                    in_=po_rows[i : i + BAND],
                )
                to_map[i] = t
            te = [te_map[i] for i in range(KER)]
            to = [to_map[i] for i in range(KER)]

            x32 = pp.tile([BAND, C * W], F32, tag="x32")
            nc.sync.dma_start(out=x32[:], in_=x32_d.ap())

            ident = pp.tile([128, 128], F16, tag="ident")
            make_identity(nc, ident[:])

            # per-partition constant columns for ACT affine (exp scale/bias)
            scol = pp.tile([128, 1], F32, tag="scol")
            nc.gpsimd.memset(scol[:], float(EXP_SCALE))
            lng = pp.tile([128, KER * KER], F32, tag="lng")
            for k in range(KER * KER):
                i, j = divmod(k, KER)
                nc.gpsimd.memset(lng[:, k : k + 1], float(math.log(GAUSS2D[i, j])))

            x08 = pp.tile([BAND, C * W], F32, tag="x08")
            nc.vector.tensor_scalar_mul(x08[:], x32[:], EDGE_PRESERVE)

            # center patch view (fp16), even alignment (offset 4)
            xc16 = te[4][:].rearrange("p (c w) -> p c w", c=C)[
                :, :, PAD : PAD + W
            ]

            outt = pp.tile([BAND, C * W], F32, tag="outt")

            for _rep in range(n_reps):
                ps = psp.tile([128, 4 * W], F32, tag="acc")  # 4 psum banks

                # ---- tap loop
                for k in range(KER * KER):
                    i, j = divmod(k, KER)
                    first = k == 0
                    last = k == KER * KER - 1
                    if j % 2 == 0:
                        tsrc, jo = te[i], j
                    else:
                        tsrc, jo = to[i], j - 1
                    p16 = tsrc[:].rearrange("p (c w) -> p c w", c=C)[
                        :, :, jo : jo + W
                    ]

                    diff = pipe.tile([BAND, C * W], F16, tag="diff")
                    d3 = diff[:].rearrange("p (c w) -> p c w", c=C)
                    nc.vector.tensor_sub(d3, p16, xc16)

                    sq = pipe.tile([BAND, C * W], F32, tag="sq")
                    if sq_engines == "act":
                        nc.scalar.activation(
                            sq[:], diff[:], mybir.ActivationFunctionType.Square
                        )
                    else:  # 2 channels on ACT, 1 on DVE
                        nc.scalar.activation(
                            sq[:, : 2 * W],
                            diff[:, : 2 * W],
                            mybir.ActivationFunctionType.Square,
                        )
                        nc.vector.tensor_mul(
                            sq[:, 2 * W :], diff[:, 2 * W :], diff[:, 2 * W :]
                        )

                    d = pipe.tile([BAND, W], F32, tag="d")
                    eng1 = nc.gpsimd if adds_gps[0] else nc.vector
                    eng2 = nc.gpsimd if adds_gps[1] else nc.vector
                    eng1.tensor_add(d[:], sq[:, :W], sq[:, W : 2 * W])
                    eng2.tensor_add(d[:], d[:], sq[:, 2 * W :])

                    w = pipe.tile([BAND, W], F16, tag="w")
                    nc.scalar.activation(
                        w[:],
                        d[:],
                        mybir.ActivationFunctionType.Exp,
                        scale=scol[:, 0:1],
                        bias=lng[:, k : k + 1],
                    )

                    prod = pipe.tile([BAND, C * W], F16, tag="prod")
                    if bcast_mul:
                        w3 = w[:].unsqueeze(1).broadcast_to((BAND, C, W))
                        nc.vector.tensor_mul(
                            prod[:].rearrange("p (c w) -> p c w", c=C), p16, w3
                        )
                    else:
                        pr3 = prod[:].rearrange("p (c w) -> p c w", c=C)
                        for c in range(C):
                            nc.vector.tensor_mul(
                                pr3[:, c], p16[:, c], w[:]
                            )

                    for c in range(C):
                        nc.tensor.matmul(
                            ps[:, c * W : (c + 1) * W],
                            ident[:],
                            prod[:, c * W : (c + 1) * W],
                            start=first,
                            stop=last,
                        )
                    nc.tensor.matmul(
                        ps[:, 3 * W : 4 * W],
                        ident[:],
                        w[:],
                        start=first,
                        stop=last,
                    )

                # ---- finish
                r = pipe.tile([BAND, W], F32, tag="recip")
                nc.vector.reciprocal(r[:], ps[:, 3 * W : 4 * W])
                t3 = pipe.tile([BAND, C * W], F32, tag="t3")
                for c in range(C):
                    nc.vector.tensor_mul(
                        t3[:, c * W : (c + 1) * W],
                        ps[:, c * W : (c + 1) * W],
                        r[:],
                    )
                nc.vector.scalar_tensor_tensor(
                    outt[:],
                    t3[:],
                    1.0 - EDGE_PRESERVE,
                    x08[:],
                    mybir.AluOpType.mult,
                    mybir.AluOpType.add,
                )
                nc.vector.tensor_scalar(
                    outt[:],
                    outt[:],
                    0.0,
                    1.0,
                    mybir.AluOpType.max,
                    mybir.AluOpType.min,
                )

            nc.sync.dma_start(out=out_d.ap(), in_=outt[:])

    nc.compile()
    return nc


# ---------------------------------------------------------------- host side
def prep_inputs(images):
    """images [2,512,512,3] fp32 -> list of 8 per-core input dicts."""
    images = np.asarray(images, dtype=np.float32)
    in_maps = []
    for b in range(B):
        xpad = np.pad(images[b], ((PAD, PAD), (PAD, PAD), (0, 0)), mode="reflect")
        xp = np.ascontiguousarray(xpad.transpose(2, 0, 1))  # [3, 520, 520]
        for v in range(H // BAND):
            band = xp[:, BAND * v : BAND * v + BAND + 2 * PAD, :]  # [3,136,520]
            pe = np.zeros((C, BAND + KER, PW), np.float16)
            pe[:, : BAND + 2 * PAD, :] = band.astype(np.float16)
            po = np.zeros((C, BAND + KER, PW), np.float16)
            po[:, : BAND + 2 * PAD, : PW - 1] = band[:, :, 1:].astype(np.float16)
            x32 = band[:, PAD : PAD + BAND, PAD : PAD + W]  # [3,128,512]
            x32 = np.ascontiguousarray(
                x32.transpose(1, 0, 2).reshape(BAND, C * W), dtype=np.float32
            )
            in_maps.append({"pe": pe, "po": po, "x32": x32})
    return in_maps


def assemble_output(results):
    """8 per-core {'out': [128, 1536]} -> [2,512,512,3] fp32."""
    full = np.empty((B, H, W, C), np.float32)
    cc = 0
    for b in range(B):
        for v in range(H // BAND):
            band = results[cc]["out"].reshape(BAND, C, W).transpose(0, 2, 1)
            full[b, BAND * v : BAND * (v + 1)] = band
            cc += 1
    return full


_NC_CACHE = {}


def get_program(n_reps=1, cfg=None):
    key = (n_reps, tuple(sorted((cfg or {}).items(), key=str)))
    if key not in _NC_CACHE:
        _NC_CACHE[key] = build_program(n_reps, cfg)
    return _NC_CACHE[key]


def run_program(nc, in_maps):
    res = bass_utils.run_bass_kernel_spmd(nc, in_maps, list(range(N_CORES)))
    return res.results


def kernel(images):
    nc = get_program(1, None)
    in_maps = prep_inputs(images)
    results = run_program(nc, in_maps)
    return assemble_output(results)


if __name__ == "__main__":
    rng = np.random.default_rng(0)
    imgs = rng.random((B, H, W, C), dtype=np.float32)
    out = kernel(imgs)
    print("out", out.shape, out.dtype, float(out.min()), float(out.max()))
